# revision 4
# baseline (speedup 1.0000x reference)
"""Multi-head causal attention (B=2, S=2048, D=4096, H=32, hd=128) on 8 trn2 cores.

Sharding: DP over batch (2) x TP over heads (4 groups of 8 heads).
Core c: batch b = c//4, head-group tp = c%4.
Each core computes a partial output [2048, 4096] (wo row-sharded); host sums
the 4 partials per batch.

Data path is bf16 (inputs rounded host-side; all matmuls accumulate in fp32
PSUM), which halves DMA traffic and lets the full x [4096, 2048] strip stay
resident in SBUF so phase 1 makes a single pass over x with weights streamed
once. Softmax/normalization arithmetic stays fp32.
Host pre-transposes x / weights so every DMA is natural-layout.
q/k head dims are de-interleaved (evens then odds) on the host so RoPE becomes
full-tile DVE ops on partition halves; the permutation is consistent between
q and k so scores are unchanged. v / wo stay in natural order.
The causal mask enters as 4 distinct [128, 512] diagonal-block patterns kept
resident in SBUF (fully-masked blocks are skipped, fully-open blocks need no
mask), so no mask bytes move during attention.
Scores are computed transposed ([tk, tq]) so the PV matmul needs no
on-chip transpose of the probabilities; softmax is unnormalized exp with the
denominator from a ones-vector matmul, divided into the attention output.
The attention block loop is software-pipelined: exp(scores) of block j+1
overlaps the PV/denominator accumulation of block j on the PE.
"""

import sys
sys.path.insert(0, '/opt/trn_rl_repo')
sys.path.insert(0, '/opt/trn_rl_repo/concourse')

import numpy as np

S = 2048
D = 4096
HD = 128
FSH = 1024            # features per core (8 heads)
NHL = 8               # heads per core
KT = D // 128         # 32 k-tiles for projections
TSTRIPS = S // 512    # 4 tq strips
NKT = S // 128        # 16 tk tiles
NEG_THRESH = -1.0e8

_cache = {}


def _build(classes, iters=0):
    """Build + compile the per-core Bacc program. classes[j][s] in {0:skip,1:zero,2:add}.

    iters=0: straight-line body (the correctness/grading path).
    iters>=1: wrap the whole body in a hardware For_i loop executing it
    `iters` times — used by test.py to measure the marginal per-iteration
    device time ((wall(N) - wall(1))/(N-1)), which cancels the fixed
    multi-ms axon dispatch overhead that dominates a single execute.
    """
    import contextlib
    import concourse.bacc as bacc
    import concourse.mybir as mybir
    import concourse.tile as tile

    f32 = mybir.dt.float32
    f32r = mybir.dt.float32r
    bf16 = mybir.dt.bfloat16
    EXP = mybir.ActivationFunctionType.Exp

    nc = bacc.Bacc("TRN2", target_bir_lowering=False, debug=False)

    xt_d = nc.dram_tensor("xt", [D, S], bf16, kind="ExternalInput").ap()
    wqt_d = nc.dram_tensor("wqt", [D, FSH], bf16, kind="ExternalInput").ap()
    wkt_d = nc.dram_tensor("wkt", [D, FSH], bf16, kind="ExternalInput").ap()
    wvt_d = nc.dram_tensor("wvt", [D, FSH], bf16, kind="ExternalInput").ap()
    wot_d = nc.dram_tensor("wot", [FSH, D], bf16, kind="ExternalInput").ap()
    cos_d = nc.dram_tensor("cosw", [64, S], f32, kind="ExternalInput").ap()
    sin_d = nc.dram_tensor("sinw", [64, S], f32, kind="ExternalInput").ap()
    nsin_d = nc.dram_tensor("nsinw", [64, S], f32, kind="ExternalInput").ap()
    maskp_d = nc.dram_tensor("maskp", [128, 4 * 512], f32, kind="ExternalInput").ap()
    id_d = nc.dram_tensor("id128", [128, 128], bf16, kind="ExternalInput").ap()
    on_d = nc.dram_tensor("ones128", [128, 128], bf16, kind="ExternalInput").ap()
    out_d = nc.dram_tensor("out", [S, D], f32, kind="ExternalOutput").ap()

    with tile.TileContext(nc) as tc, \
         nc.allow_low_precision(reason="bf16 data path, fp32 accumulation"), \
         (tc.For_i(0, iters, 1) if iters else contextlib.nullcontext()):
        with tc.tile_pool(name="pdram", bufs=1, space="DRAM") as pdram, \
             tc.tile_pool(name="pconst", bufs=1) as pconst:
            qt_d = pdram.tile([FSH, S], bf16, name="qt_spill")
            kt_d = pdram.tile([FSH, S], bf16, name="kt_spill")
            vt_d = pdram.tile([FSH, S], bf16, name="vt_spill")
            ones_sb = pconst.tile([128, 128], bf16, name="ones_sb")
            nc.gpsimd.dma_start(out=ones_sb, in_=on_d)
            id_sb = pconst.tile([128, 128], bf16, name="id_sb")
            nc.gpsimd.dma_start(out=id_sb, in_=id_d)
            maskp_sb = pconst.tile([128, 4 * 512], f32, name="maskp_sb")
            nc.gpsimd.dma_start(out=maskp_sb, in_=maskp_d)
            ones_k = ones_sb[:, 0:1]

            # ---------------- Phase 1: q/k/v projections (+RoPE on q,k) -------------
            # Full x [D, S] (16 MiB bf16) streams into SBUF once; the 24
            # (projection, head) jobs each stream their weight tile once and
            # contract against the resident x. The first W jobs run in
            # wavefront (k-major) order so the PE starts as soon as the first
            # x k-tiles land instead of waiting for the whole stream.
            with tc.tile_pool(name="p1x", bufs=KT) as p1x, \
                 tc.tile_pool(name="p1w", bufs=3) as p1w, \
                 tc.tile_pool(name="p1t", bufs=4) as p1t, \
                 tc.tile_pool(name="p1o", bufs=4) as p1o, \
                 tc.tile_pool(name="p1cs", bufs=1) as p1cs, \
                 tc.tile_pool(name="ps1", bufs=8, space="PSUM") as ps1:
                w_ds = [wqt_d, wkt_d, wvt_d]
                spills = [qt_d, kt_d, vt_d]
                jobs = [(proj, i) for proj in range(3) for i in range(NHL)]

                def load_w(proj, i):
                    wt = p1w.tile([128, KT, 128], bf16, name="wt")
                    w_ap = w_ds[proj][:, i * 128:(i + 1) * 128].rearrange(
                        "(k p) f -> p k f", p=128)
                    nc.sync.dma_start(out=wt, in_=w_ap)
                    return wt

                W = 2                     # wavefront width (W*4 PSUM banks)
                wt_wave = [load_w(*jobs[w]) for w in range(W)]

                xk = []
                for k in range(KT):
                    xt_t = p1x.tile([128, S], bf16, name="xk")
                    nc.scalar.dma_start(out=xt_t, in_=xt_d[k * 128:(k + 1) * 128, :])
                    xk.append(xt_t)
                cos_sb = p1cs.tile([64, S], f32, name="cos_sb")
                sin_sb = p1cs.tile([64, S], f32, name="sin_sb")
                nsin_sb = p1cs.tile([64, S], f32, name="nsin_sb")
                nc.gpsimd.dma_start(out=cos_sb, in_=cos_d)
                nc.gpsimd.dma_start(out=sin_sb, in_=sin_d)
                nc.gpsimd.dma_start(out=nsin_sb, in_=nsin_d)

                def finish_chunk(proj, i, ps, t4):
                    """RoPE (q,k) or copy (v) one [128,512] PSUM chunk and spill."""
                    ot = p1o.tile([128, 512], bf16, name="ot")
                    csl = slice(t4 * 512, (t4 + 1) * 512)
                    if proj < 2:  # RoPE for q, k
                        m1 = p1t.tile([64, 512], f32, name="m1")
                        m2 = p1t.tile([64, 512], f32, name="m2")
                        nc.vector.tensor_mul(m1, ps[0:64], cos_sb[:, csl])
                        nc.vector.tensor_mul(m2, ps[64:128], nsin_sb[:, csl])
                        nc.vector.tensor_add(ot[0:64], m1, m2)
                        m3 = p1t.tile([64, 512], f32, name="m1")
                        m4 = p1t.tile([64, 512], f32, name="m2")
                        nc.vector.tensor_mul(m3, ps[0:64], sin_sb[:, csl])
                        nc.vector.tensor_mul(m4, ps[64:128], cos_sb[:, csl])
                        nc.vector.tensor_add(ot[64:128], m3, m4)
                    else:
                        nc.vector.tensor_copy(ot, ps)
                    nc.gpsimd.dma_start(
                        out=spills[proj][i * 128:(i + 1) * 128,
                                         t4 * 512:(t4 + 1) * 512],
                        in_=ot)

                # wavefront pass: jobs[0:W], k-major
                pss = [[ps1.tile([128, 512], f32, name="ps1") for _ in range(4)]
                       for _ in range(W)]
                for k in range(KT):
                    for w in range(W):
                        for t4 in range(4):
                            nc.tensor.matmul(
                                pss[w][t4], wt_wave[w][:, k, :],
                                xk[k][:, t4 * 512:(t4 + 1) * 512],
                                start=(k == 0), stop=(k == KT - 1))
                wt_next = load_w(*jobs[W])
                for w in range(W):
                    for t4 in range(4):
                        finish_chunk(*jobs[w], pss[w][t4], t4)

                # dense pass: jobs[W:], x fully resident by now
                for idx in range(W, len(jobs)):
                    proj, i = jobs[idx]
                    wt = wt_next
                    if idx + 1 < len(jobs):
                        wt_next = load_w(*jobs[idx + 1])
                    for t4 in range(4):
                        ps = ps1.tile([128, 512], f32, name="ps1")
                        for k in range(KT):
                            nc.tensor.matmul(
                                ps, wt[:, k, :],
                                xk[k][:, t4 * 512:(t4 + 1) * 512],
                                start=(k == 0), stop=(k == KT - 1))
                        finish_chunk(proj, i, ps, t4)

            # ---------------- Phase 2: attention per head ----------------------------
            with tc.tile_pool(name="patt", bufs=1) as patt, \
                 tc.tile_pool(name="p3w", bufs=2) as p3w:
              att_sb = [patt.tile([128, S], bf16, name=f"attT{h}") for h in range(NHL)]
              with tc.tile_pool(name="p2h", bufs=2) as p2h, \
                   tc.tile_pool(name="p2v", bufs=2 * NKT + 1) as p2v, \
                   tc.tile_pool(name="p2e", bufs=6) as p2e, \
                   tc.tile_pool(name="p2ms", bufs=3) as p2ms, \
                   tc.tile_pool(name="p2r", bufs=4) as p2r, \
                   tc.tile_pool(name="p2o", bufs=4) as p2o, \
                   tc.tile_pool(name="ps2s", bufs=2, space="PSUM") as ps2s, \
                   tc.tile_pool(name="ps2a", bufs=2, space="PSUM") as ps2a, \
                   tc.tile_pool(name="ps2t", bufs=2, space="PSUM") as ps2t, \
                   tc.tile_pool(name="ps2d", bufs=2, space="PSUM") as ps2d:
                  for h in range(NHL):
                      vt_h = p2h.tile([128, S], bf16, name="vt_h")
                      kt_h = p2h.tile([128, S], bf16, name="kt_h")
                      qt_h = p2h.tile([128, S], bf16, name="qt_h")
                      nc.sync.dma_start(out=vt_h, in_=vt_d[h * 128:(h + 1) * 128, :])
                      nc.sync.dma_start(out=kt_h, in_=kt_d[h * 128:(h + 1) * 128, :])
                      nc.sync.dma_start(out=qt_h, in_=qt_d[h * 128:(h + 1) * 128, :])
                      v_sb = []
                      for j in range(NKT):
                          tps = ps2t.tile([128, 128], f32r, name="tp")
                          nc.tensor.transpose(tps, vt_h[:, j * 128:(j + 1) * 128], id_sb)
                          vj = p2v.tile([128, 128], bf16, name="vj")
                          nc.vector.tensor_copy(vj, tps)
                          v_sb.append(vj)
                      for s in range(TSTRIPS):
                          act = [j for j in range(NKT) if classes[j][s] != 0]
                          A = ps2a.tile([128, 512], f32, name="A")
                          Dn = ps2d.tile([1, 512], f32, name="Dn")
                          qs = qt_h[:, s * 512:(s + 1) * 512]

                          def emit_scores(j):
                              sps = ps2s.tile([128, 512], f32, name="sps")
                              nc.tensor.matmul(sps, kt_h[:, j * 128:(j + 1) * 128],
                                               qs, start=True, stop=True)
                              E = p2e.tile([128, 512], bf16, name="E")
                              if classes[j][s] == 2:
                                  p = j - 4 * s
                                  ms = p2ms.tile([128, 512], f32, name="ms")
                                  nc.vector.tensor_add(
                                      ms, sps, maskp_sb[:, p * 512:(p + 1) * 512])
                                  nc.scalar.activation(E, ms, EXP)
                              else:
                                  nc.scalar.activation(E, sps, EXP)
                              return E

                          E_cur = emit_scores(act[0])
                          for idx, j in enumerate(act):
                              E_next = (emit_scores(act[idx + 1])
                                        if idx + 1 < len(act) else None)
                              first, last = (idx == 0), (idx == len(act) - 1)
                              nc.tensor.matmul(A, v_sb[j], E_cur,
                                               start=first, stop=last)
                              nc.tensor.matmul(Dn, ones_k, E_cur,
                                               start=first, stop=last)
                              E_cur = E_next
                          rec = p2r.tile([1, 512], f32r, name="rec")
                          nc.vector.reciprocal(rec, Dn[0:1, :])
                          bsb = p2o.tile([128, 512], f32r, name="bsb")
                          nc.gpsimd.partition_broadcast(bsb, rec, 128)
                          nc.vector.tensor_mul(
                              att_sb[h][:, s * 512:(s + 1) * 512], A, bsb)

              # ---------------- Phase 3: output projection ------------------------------
              if True:
                with tc.tile_pool(name="p3o", bufs=4) as p3o, \
                   tc.tile_pool(name="ps3", bufs=4, space="PSUM") as ps3:
                  def load_w3(c):
                      wt = p3w.tile([128, NHL, 512], bf16, name="w3")
                      w_ap = wot_d[:, c * 512:(c + 1) * 512].rearrange(
                          "(k p) f -> p k f", p=128)
                      nc.sync.dma_start(out=wt, in_=w_ap)
                      return wt

                  wt_next3 = load_w3(0)
                  am = att_sb
                  for c in range(8):        # dout chunks of 512
                      wt = wt_next3
                      if c + 1 < 8:
                          wt_next3 = load_w3(c + 1)
                      for m in range(NKT):  # t tiles of 128
                          ps = ps3.tile([128, 512], f32, name="ps3")
                          for k in range(NHL):
                              nc.tensor.matmul(ps, am[k][:, m * 128:(m + 1) * 128],
                                               wt[:, k, :],
                                               start=(k == 0), stop=(k == NHL - 1))
                          ot = p3o.tile([128, 512], f32, name="o3")
                          nc.vector.tensor_copy(ot, ps)
                          nc.gpsimd.dma_start(
                              out=out_d[m * 128:(m + 1) * 128, c * 512:(c + 1) * 512],
                              in_=ot)

    nc.compile()
    return nc


def _host_prep(x, wq, wk, wv, wo, freqs_cos, freqs_sin, mask):
    """Build per-core input maps + mask block classes."""
    import ml_dtypes
    bf16 = ml_dtypes.bfloat16

    x = np.asarray(x, np.float32)
    wq = np.asarray(wq, np.float32)
    wk = np.asarray(wk, np.float32)
    wv = np.asarray(wv, np.float32)
    wo = np.asarray(wo, np.float32)
    mask2 = np.asarray(mask, np.float32).reshape(S, S)

    perm = np.concatenate(
        [hl * 128 + np.concatenate([np.arange(0, 128, 2), np.arange(1, 128, 2)])
         for hl in range(NHL)])
    cosw = np.ascontiguousarray(np.asarray(freqs_cos, np.float32).T)
    sinw = np.ascontiguousarray(np.asarray(freqs_sin, np.float32).T)
    nsinw = np.ascontiguousarray(-sinw)
    maskt = np.ascontiguousarray(mask2.T)
    id128 = np.eye(128, dtype=np.float32)

    classes = [[0] * TSTRIPS for _ in range(NKT)]
    for j in range(NKT):
        for s in range(TSTRIPS):
            blk = maskt[j * 128:(j + 1) * 128, s * 512:(s + 1) * 512]
            if (blk <= NEG_THRESH).all():
                classes[j][s] = 0
            elif (blk == 0.0).all():
                classes[j][s] = 1
            else:
                classes[j][s] = 2

    # The partially-masked blocks of a causal mask come in exactly 4 shapes
    # (offset of the 128-row k-block within its 512-col q-strip); keep those
    # resident instead of streaming mask bytes. Verify the assumption holds
    # for the mask we were actually given.
    maskp = np.zeros((128, 4 * 512), np.float32)
    for p in range(4):
        maskp[:, p * 512:(p + 1) * 512] = maskt[p * 128:(p + 1) * 128, 0:512]
    for j in range(NKT):
        for s in range(TSTRIPS):
            if classes[j][s] == 2:
                p = j - 4 * s
                assert 0 <= p < 4, (j, s)
                assert np.array_equal(
                    maskt[j * 128:(j + 1) * 128, s * 512:(s + 1) * 512],
                    maskp[:, p * 512:(p + 1) * 512]), (j, s)

    xts = [np.ascontiguousarray(x[b].T).astype(bf16) for b in range(2)]
    in_maps = []
    for core in range(8):
        b, tp = core // 4, core % 4
        sl = slice(tp * FSH, (tp + 1) * FSH)
        wq_c = wq[sl][perm] * np.float32(1.0 / np.sqrt(HD))
        wk_c = wk[sl][perm]
        in_maps.append({
            "xt": xts[b],
            "wqt": np.ascontiguousarray(wq_c.T).astype(bf16),
            "wkt": np.ascontiguousarray(wk_c.T).astype(bf16),
            "wvt": np.ascontiguousarray(wv[sl].T).astype(bf16),
            "wot": np.ascontiguousarray(wo[:, sl].T).astype(bf16),
            "cosw": cosw, "sinw": sinw, "nsinw": nsinw,
            "maskp": maskp, "id128": id128.astype(bf16),
            "ones128": np.ones((128, 128), bf16),
        })
    return in_maps, classes


def kernel(x, wq, wk, wv, wo, freqs_cos, freqs_sin, mask, start_pos=0,
           _trace=False):
    from concourse import bass_utils
    in_maps, classes = _host_prep(x, wq, wk, wv, wo, freqs_cos, freqs_sin, mask)
    key = str(classes)
    if key not in _cache:
        _cache[key] = _build(classes)
    nc = _cache[key]
    res = bass_utils.run_bass_kernel_spmd(nc, in_maps, core_ids=list(range(8)),
                                          trace=_trace)
    out = np.zeros((2, S, D), np.float32)
    for core in range(8):
        out[core // 4] += res.results[core]["out"]
    kernel.last_result = res
    return out


if __name__ == "__main__":
    # compile-only smoke test
    classes = [[2 if j * 128 <= s * 512 + 511 and j * 128 + 127 > s * 512 else
                (1 if j * 128 + 127 <= s * 512 else 0)
                for s in range(TSTRIPS)] for j in range(NKT)]
    import time
    t0 = time.time()
    nc = _build(classes)
    print(f"build+bacc-compile: {time.time()-t0:.1f}s")
    from concourse.timeline_sim import TimelineSim
    est = TimelineSim(nc, trace=False).simulate()
    print(f"TimelineSim: {est:.0f} ns")
    if len(sys.argv) > 1 and sys.argv[1] == "neff":
        import tempfile
        from concourse import bass_utils
        t0 = time.time()
        with tempfile.TemporaryDirectory() as td:
            bass_utils.compile_bass_kernel(nc, td)
            print(f"walrus: {time.time()-t0:.1f}s COMPILED OK")
